# revision 52
# baseline (speedup 1.0000x reference)
"""Distributed Trainium2 attention kernel (8 NeuronCores, head tensor-parallel).

Reference semantics (T=4096, D=2048, H=16, DH=128):
  qkv = bf16(x @ W_qkv); q,k,v per head; RoPE(split-half) on q,k;
  mask = ((m_q & m_k) | eye) & causal; softmax(q k^T / sqrt(DH) masked);
  out = bf16((probs @ v) @ W_out)

Sharding: head tensor-parallel for qkv+SDPA (core c owns heads 2c, 2c+1),
then an AllToAll redistributes the small per-head attention outputs o so
that each core owns 64 output ROWS per 512-query quad (512 rows total)
and computes the full out-projection locally against a replicated W_out.
This moves 8x fewer bytes than reduce-scattering output partials.

Device-side layout choices:
  - x passed as xT [D, T] so the D contraction dim is the partition dim.
  - q,k computed weight-stationary -> born transposed [DH, T]; v
    transposed back to natural [T, DH] via PE (PV lhsT layout).
  - RoPE: partition-rotate by 64 via two SBUF->SBUF DMAs, sign folded
    into a host-precomputed ssinT table; combine on DVE.
  - SDPA in transposed-scores form: scoresT[k, q] tiles over 512-query
    quads; exp (no max-subtraction; scores are O(5)) evacuates the
    scores psum straight into the PV rhs; key padding mask folded into
    the exp bias (per-k = per-partition); within-block causal via one
    precomputed 0/1 [128,128] multiply; softmax denominators via a
    ones-column matmul.
  - Denominators: non-diagonal exp blocks are tree-summed on DVE in
    bf16 (pairs -> 4-groups -> 8-groups, ~0.3% on the sum) so the PE
    ones-matmul runs once per up-to-8 blocks; diagonal blocks keep
    per-block ones-matmuls (they carry the partial-column offsets).
  - Normalization fused into the oT evacuation: inv = m_q/(den+(1-m_q))
    computed on one partition, broadcast to all 128 partitions with a
    K=1 ones-outer-product matmul on PE, then oT = oraw*bc + vT*(1-m_q)
    (vT premultiplied by (1-m_q) once). Masked queries thereby attend
    only to themselves; out-projection needs no per-tile scaling.
  - Engine split in phase 2: ACT does exp only; DVE does all psum
    evacuations, the denominator tree and the inv chain; bc matmuls and
    evacuations lag one head-sequence behind the PE block stream so the
    PE never waits on the DVE chain.
  - A2A per quad PAIR (512KB) for pairs 0-2; the final pair is split
    into per-quad A2As so quad 6's half flies during quad 7's SDPA and
    the exposed tail collective carries only 256KB.  Two tiny warm-up
    A2As (kernel start + phase boundary) absorb CC-path init and rank
    skew.  Collective-dependent loads ride the GpSimd DMA queue so they
    cannot head-of-line-block the Sync queue; bulk input loads also use
    the GpSimd SWDGE ring, which sustains far higher bandwidth than the
    HWDGE rings.
  - Out-projections are lag-scheduled behind their A2As.  The exp on
    ACT (686ns/block) slightly outpaces the PE block stream (~620ns),
    so pair 0's projection is WOVEN into the SDPA stream one matmul per
    two blocks, executing inside that per-block PE slack; pair 2's
    projection is deferred entirely into the final A2A's rendezvous
    window; pair 3 is the unavoidable post-collective tail.
"""

import os
import sys

import numpy as np

sys.path.insert(0, "/opt/trn_rl_repo")

import ml_dtypes

BF16 = ml_dtypes.bfloat16

# problem constants (hardcoded per harness contract)
T, D, H, DH = 4096, 2048, 16, 128
N_CORES = 8
ROPE_BASE = 10000.0


def build_nc(
    t=T,
    d=D,
    n_cores=N_CORES,
    hl=H // N_CORES,  # heads per core
    tch=512,  # qkv t-chunk
):
    import concourse.bass as bass
    import concourse.mybir as mybir
    import concourse.tile as tile
    from concourse import bacc
    from concourse.masks import make_identity

    f32 = mybir.dt.float32
    bf16 = mybir.dt.bfloat16

    P = 128
    kd = d // P  # contraction chunks for qkv
    qb_n = t // P  # q-blocks of 128 rows
    nt = t // tch  # t-chunks in qkv phase
    qw = 512  # queries per quad
    n_quads = t // qw
    qb_per_quad = qw // P  # 4
    rows_per_rank = qw // n_cores  # 64 rows each rank owns per quad
    n_pairs = n_quads // 2
    t_out = t // n_cores  # output rows per core
    scale = 1.0 / np.sqrt(DH)

    nc = bacc.Bacc(
        "TRN2", target_bir_lowering=False, debug=False, num_devices=n_cores
    )

    xT = nc.dram_tensor("xT", [d, t], bf16, kind="ExternalInput").ap()
    # wqkv pre-transposed on host to partition-major [P, c, k, j] so
    # per-channel weight DMAs have 4KB-contiguous lines per partition
    wqkv = nc.dram_tensor("wqkv", [P, 3 * hl * kd * P], bf16, kind="ExternalInput").ap()
    wout_d = nc.dram_tensor("wout", [d, d], bf16, kind="ExternalInput").ap()
    cosT_d = nc.dram_tensor("cosT", [P, t], f32, kind="ExternalInput").ap()
    ssinT_d = nc.dram_tensor("ssinT", [P, t], f32, kind="ExternalInput").ap()
    # rqT[p, qb] = 0 if mask[qb*128+p] else -1e9 (folded into exp bias)
    rqT_d = nc.dram_tensor("rqT", [P, qb_n], f32, kind="ExternalInput").ap()
    # dvalB[p, q] = 1 - mask[q], broadcast to all partitions
    dvalB_d = nc.dram_tensor("dvalB", [P, t], bf16, kind="ExternalInput").ap()
    # mrow[0, q] = mask[q] ; dvalrow[0, q] = 1 - mask[q]
    mrow_d = nc.dram_tensor("mrow", [1, t], bf16, kind="ExternalInput").ap()
    dvalrow_d = nc.dram_tensor("dvalrow", [1, t], bf16, kind="ExternalInput").ap()
    # cmask128[p, j] = 1 if j >= p else 0 (within-block causal, T-orientation)
    cmask128_d = nc.dram_tensor("cmask128", [P, P], bf16, kind="ExternalInput").ap()
    out_d = nc.dram_tensor("out", [t_out, d], bf16, kind="ExternalOutput").ap()

    with tile.TileContext(nc) as tc:
        with tc.tile_pool(name="persist", bufs=1) as persist:
            ident = persist.tile([P, P], bf16, name="ident")
            ones_col = persist.tile([P, 1], bf16, name="ones_col")
            nc.vector.memset(ones_col, 1.0)
            ones_row = persist.tile([1, P], bf16, name="ones_row")
            nc.vector.memset(ones_row, 1.0)
            rqT_sb = persist.tile([P, qb_n], f32, name="rqT_sb")
            cm128_sb = persist.tile([P, P], bf16, name="cm128_sb")
            mrow_sb = persist.tile([1, t], bf16, name="mrow_sb")
            dvalrow_sb = persist.tile([1, t], bf16, name="dvalrow_sb")

            def load_masks():
                # deferred: these tiny loads would otherwise delay the x
                # stream at the head of the gpsimd DMA ring
                nc.gpsimd.dma_start(rqT_sb, rqT_d)
                nc.gpsimd.dma_start(cm128_sb, cmask128_d)
                nc.gpsimd.dma_start(mrow_sb, mrow_d)
                nc.gpsimd.dma_start(dvalrow_sb, dvalrow_d)

            with tc.tile_pool(name="dram_warm", bufs=1, space="DRAM") as dwarm:
                warm_in = dwarm.tile([n_cores * 16, 16], bf16, name="cc_warm_in")
                warm_out = dwarm.tile([n_cores * 16, 16], bf16, name="cc_warm_out")

            # per-head persistent activations
            qT = [persist.tile([P, t], bf16, name=f"qT{h}") for h in range(hl)]
            kT = [persist.tile([P, t], bf16, name=f"kT{h}") for h in range(hl)]
            vT = [persist.tile([P, t], bf16, name=f"vT{h}") for h in range(hl)]
            v_nat = [
                persist.tile([P, qb_n, P], bf16, name=f"vnat{h}") for h in range(hl)
            ]

            # ---------------- phase 1: qkv + rope + v transpose ----------
            with (
                tc.tile_pool(name="wq", bufs=1) as wqpool,
                tc.tile_pool(name="cs", bufs=1) as cspool,
                tc.tile_pool(name="ph1", bufs=2) as ph1,
                tc.tile_pool(name="ps_qkv", bufs=1, space="PSUM") as ps_qkv,
                tc.tile_pool(name="ps_aux", bufs=2, space="PSUM") as ps_aux,
            ):
                cosT_sb = cspool.tile([P, t], f32, name="cosT_sb")
                ssinT_sb = cspool.tile([P, t], f32, name="ssinT_sb")
                dvalB_sb = cspool.tile([P, t], bf16, name="dvalB_sb")
                # rope tables on the gpsimd (SWDGE) ring, first two
                # t-chunks upfront, the rest paced inside the loop so they
                # don't steal DMA bandwidth from the x stream at startup
                def load_tables(tc_i):
                    tsl = slice(tc_i * tch, (tc_i + 1) * tch)
                    nc.gpsimd.dma_start(cosT_sb[:, tsl], cosT_d[:, tsl])
                    nc.gpsimd.dma_start(ssinT_sb[:, tsl], ssinT_d[:, tsl])



                # fire a tiny throwaway AllToAll after the bulk loads: the
                # first collective pays the CC-path init + rank-sync cost
                # (~100us on the baseline's first ReduceScatter), which
                # this hides under the qkv phase without delaying the
                # gpsimd DMA ring; a second one at the end of phase 1
                # re-syncs rank skew just before the real A2As
                nc.gpsimd.collective_compute(
                    "AllToAll",
                    mybir.AluOpType.bypass,
                    replica_groups=[list(range(n_cores))],
                    ins=[warm_in.opt()],
                    outs=[warm_out.opt()],
                )

                # weights channel-major so channel 0's 16 k-chunks land
                # first and the very first matmul group isn't DMA-starved
                wq_sb = wqpool.tile([P, 3 * hl, kd, P], bf16, name="wq_sb")
                wqkv_r = wqkv.rearrange("p (c kd j) -> p c kd j", kd=kd, j=P)
                xT_r = xT.rearrange("(kd p) x -> p kd x", p=P)

                xts = {}

                def load_xt(tc_i):
                    # x rides the gpsimd (SWDGE) ring, which empirically
                    # sustains ~250GB/s vs ~50GB/s on the HWDGE rings
                    xt = ph1.tile([P, kd, tch], bf16, tag="xt")
                    tsl = slice(tc_i * tch, (tc_i + 1) * tch)
                    gw = 2 if tc_i == 0 else 4
                    for k4 in range(kd // gw):
                        ksl = slice(k4 * gw, (k4 + 1) * gw)
                        nc.gpsimd.dma_start(xt[:, ksl], xT_r[:, ksl, tsl])
                    nc.gpsimd.dma_start(dvalB_sb[:, tsl], dvalB_d[:, tsl])
                    if tc_i == 0:
                        # weights ride the sync ring, streaming in
                        # parallel with the x chunks on the gpsimd ring
                        for c in range(3 * hl):
                            nc.sync.dma_start(wq_sb[:, c], wqkv_r[:, c])
                    xts[tc_i] = xt

                pending_vt = []

                def flush_vt():
                    h, b0, b1 = pending_vt.pop(0)
                    for b in range(b0, b1):
                        pst = ps_aux.tile([P, P], bf16, tag="aux")
                        nc.tensor.transpose(
                            pst, vT[h][:, b * P : (b + 1) * P], ident
                        )
                        nc.scalar.copy(v_nat[h][:, b], pst)
                    # vT only feeds the masked-query blend once transposed:
                    # premultiply this chunk by (1 - m_q) in place
                    csl = slice(b0 * P, b1 * P)
                    nc.vector.tensor_tensor(
                        vT[h][:, csl], vT[h][:, csl], dvalB_sb[:, csl],
                        mybir.AluOpType.mult,
                    )

                load_xt(0)
                load_tables(0)
                load_tables(1)
                # identity for the PE transposes (gpsimd memset+select is
                # slow on Q7 -- keep it behind the startup x/weight DMAs)
                make_identity(nc, ident)
                for tc_i in range(nt):
                    tsl = slice(tc_i * tch, (tc_i + 1) * tch)
                    if tc_i + 1 < nt:
                        load_xt(tc_i + 1)  # prefetch one chunk ahead
                    if tc_i + 2 < nt:
                        load_tables(tc_i + 2)
                    if tc_i == 2:
                        load_masks()
                    xt = xts.pop(tc_i)
                    for c in range(3 * hl):  # q0,q1,k0,k1,v0,v1
                        ps = ps_qkv.tile([P, tch], mybir.dt.float32, tag=f"ps{c}")
                        for k in range(kd):
                            nc.tensor.matmul(
                                ps,
                                lhsT=wq_sb[:, c, k],
                                rhs=xt[:, k],
                                start=(k == 0),
                                stop=(k == kd - 1),
                            )
                        if pending_vt:
                            # v transposes lag one channel behind so the PE
                            # never waits on the vT psum->sbuf cast
                            flush_vt()
                        if c < 2 * hl:  # q or k: cast, rotate, rope-combine
                            dst = qT[c] if c < hl else kT[c - hl]
                            qbf = ph1.tile([P, tch], bf16, tag="qbf")
                            nc.scalar.copy(qbf, ps)
                            # rotate-half: partition shift by 64 via two
                            # SBUF->SBUF DMAs (keeps PE free; scalar queue
                            # so the sync queue keeps streaming x chunks)
                            shift = ph1.tile([P, tch], bf16, tag="shift")
                            nc.scalar.dma_start(shift[0:64], qbf[64:128])
                            nc.scalar.dma_start(shift[64:128], qbf[0:64])
                            t1 = ph1.tile([P, tch], f32, tag="t1")
                            nc.vector.tensor_tensor(
                                t1, qbf, cosT_sb[:, tsl], mybir.AluOpType.mult
                            )
                            t2 = ph1.tile([P, tch], f32, tag="t2")
                            nc.vector.tensor_tensor(
                                t2, shift, ssinT_sb[:, tsl], mybir.AluOpType.mult
                            )
                            nc.vector.tensor_tensor(
                                dst[:, tsl], t1, t2, mybir.AluOpType.add
                            )
                        else:  # v: cast; queue this chunk's transposes
                            h = c - 2 * hl
                            nc.scalar.copy(vT[h][:, tsl], ps)
                            pending_vt.append(
                                (h, tc_i * tch // P, (tc_i + 1) * tch // P)
                            )
                while pending_vt:
                    flush_vt()
                nc.gpsimd.collective_compute(
                    "AllToAll",
                    mybir.AluOpType.bypass,
                    replica_groups=[list(range(n_cores))],
                    ins=[warm_in.opt()],
                    outs=[warm_out.opt()],
                )

            # ---------------- phase 2: SDPA + A2A + out-proj -------------
            with (
                tc.tile_pool(name="w2", bufs=1) as w2pool,
                tc.tile_pool(name="ph2", bufs=2) as ph2,
                tc.tile_pool(name="pt", bufs=6) as ptpool,
                tc.tile_pool(name="pair", bufs=5) as pairpool,
                tc.tile_pool(name="lhsp", bufs=2) as lhsppool,
                tc.tile_pool(name="dram", bufs=1, space="DRAM") as dram,
                tc.tile_pool(name="ps_s", bufs=3, space="PSUM") as ps_s,
                tc.tile_pool(name="ps_o", bufs=2, space="PSUM") as ps_o,
                tc.tile_pool(name="ps_d", bufs=1, space="PSUM") as ps_d,
                tc.tile_pool(name="ps_out", bufs=2, space="PSUM") as ps_out,
            ):
                wout_sb = w2pool.tile([P, kd, d], bf16, name="wout_sb")
                nc.gpsimd.dma_start(
                    wout_sb, wout_d.rearrange("(h p) x -> p h x", p=P)
                )

                # one A2A per quad PAIR (512KB) instead of per quad:
                # halves the latency-bound collective-pipe occupancy.
                # The final pair is split back into per-quad A2As so quad
                # 6's half flies during quad 7's SDPA and the exposed
                # tail collective carries only 256KB.
                a2a_in = [
                    dram.tile([hl * n_cores * P, 2 * rows_per_rank], bf16,
                              name=f"a2a_in{p}")
                    for p in range(n_pairs - 1)
                ]
                a2a_out = [
                    dram.tile([hl * n_cores * P, 2 * rows_per_rank], bf16,
                              name=f"a2a_out{p}")
                    for p in range(n_pairs - 1)
                ]
                a2a_in_q = {
                    g: dram.tile([hl * n_cores * P, rows_per_rank], bf16,
                                 name=f"a2a_inq{g}")
                    for g in (n_quads - 2, n_quads - 1)
                }
                a2a_out_q = {
                    g: dram.tile([hl * n_cores * P, rows_per_rank], bf16,
                                 name=f"a2a_outq{g}")
                    for g in (n_quads - 2, n_quads - 1)
                }

                lhs_tiles = {}
                weave_q = []
                blk_ctr = [0]

                def push_outproj_weave(p):
                    """Queue pair p's out-projection as fine-grained items
                    woven one-per-two SDPA blocks, consuming the PE slack
                    left by the exp-bound block stream."""
                    lhsP = lhs_tiles.pop(p)
                    partial = ph2.tile([P, d], bf16, tag="partial",
                                       name="partial")
                    open_ps = {}
                    for ntile in range(d // 512):
                        nsl = slice(ntile * 512, (ntile + 1) * 512)
                        for j in range(kd):
                            def mm(j=j, ntile=ntile, nsl=nsl):
                                if j == 0:
                                    open_ps[ntile] = ps_out.tile(
                                        [P, 512], mybir.dt.float32,
                                        tag="outps", name="outps_w"
                                    )
                                nc.tensor.matmul(
                                    open_ps[ntile],
                                    lhsT=lhsP[:, j],
                                    rhs=wout_sb[:, j, nsl],
                                    start=(j == 0),
                                    stop=(j == kd - 1),
                                )
                            weave_q.append(mm)
                        def evac(ntile=ntile, nsl=nsl):
                            nc.vector.tensor_copy(
                                partial[:, nsl], open_ps.pop(ntile)
                            )
                            nc.sync.dma_start(
                                out_d[p * P : (p + 1) * P, nsl], partial[:, nsl]
                            )
                        weave_q.append(evac)

                def emit_sdpa_blocks(g, h):
                    """PE block stream for one (quad, head): scores, exp,
                    PV + denominator accumulate.  Returns psum handles.

                    Denominator: non-diagonal blocks are pair-summed on DVE
                    (bf16, ~0.1% on the sum) so the PE ones-matmul runs once
                    per pair; diagonal blocks keep per-block ones-matmuls."""
                    nsk = (g + 1) * qb_per_quad
                    diag_start = g * qb_per_quad
                    pso = ps_o.tile([P, qw], mybir.dt.float32, tag="pso")
                    psd = ps_d.tile([1, qw], mybir.dt.float32, tag="psd")
                    # non-diag blocks are tree-summed on DVE (pairs ->
                    # 4-groups -> 8-groups) so the PE ones-matmul runs once
                    # per up-to-8 blocks; ~0.3% on the denominator, far
                    # inside the error budget
                    ndiag = diag_start
                    n_psd = ndiag // 8 + (1 if ndiag % 8 else 0) + qb_per_quad
                    psd_state = [0]
                    pend_pairs = []
                    pend_g4 = []
                    pend_groups = []

                    def emit_psd(rhs_ap, lo):
                        nc.tensor.matmul(
                            psd[:, lo:],
                            lhsT=ones_col,
                            rhs=rhs_ap,
                            start=(psd_state[0] == 0),
                            stop=(psd_state[0] == n_psd - 1),
                        )
                        psd_state[0] += 1

                    def emit_score(sk):
                        br = sk - diag_start  # >=0 in diag region
                        lo = br * P if br >= 0 else 0
                        psT = ps_s.tile([P, qw], mybir.dt.float32, tag="scT")
                        nc.tensor.matmul(
                            psT[:, lo:],
                            lhsT=kT[h][:, sk * P : (sk + 1) * P],
                            rhs=qT[h][:, g * qw + lo : (g + 1) * qw],
                            start=True,
                            stop=True,
                        )
                        pT = ptpool.tile([P, qw], bf16, tag="pT")
                        nc.scalar.activation(
                            pT[:, lo:],
                            psT[:, lo:],
                            mybir.ActivationFunctionType.Exp,
                            scale=float(scale),
                            bias=rqT_sb[:, sk : sk + 1],
                        )
                        if br >= 0:
                            nc.vector.tensor_tensor(
                                pT[:, lo : lo + P],
                                pT[:, lo : lo + P],
                                cm128_sb,
                                mybir.AluOpType.mult,
                            )
                        return pT, lo

                    prev_pT = [None]

                    def emit_pv(sk, pT, lo):
                        nc.tensor.matmul(
                            pso[:, lo:],
                            lhsT=v_nat[h][:, sk],
                            rhs=pT[:, lo:],
                            start=(sk == 0),
                            stop=(sk == nsk - 1),
                        )
                        if sk < diag_start:
                            if sk % 2 == 0:
                                prev_pT[0] = pT
                            else:
                                pairT = pairpool.tile([P, qw], bf16, tag="pairT")
                                nc.vector.tensor_tensor(
                                    pairT, prev_pT[0], pT, mybir.AluOpType.add
                                )
                                pend_pairs.append(pairT)
                                if len(pend_pairs) == 2:
                                    b = pend_pairs.pop()
                                    a = pend_pairs.pop()
                                    grpT = pairpool.tile(
                                        [P, qw], bf16, tag="grpT"
                                    )
                                    nc.vector.tensor_tensor(
                                        grpT, a, b, mybir.AluOpType.add
                                    )
                                    pend_g4.append(grpT)
                                    if len(pend_g4) == 2:
                                        b4 = pend_g4.pop()
                                        a4 = pend_g4.pop()
                                        g8T = pairpool.tile(
                                            [P, qw], bf16, tag="g8T"
                                        )
                                        nc.vector.tensor_tensor(
                                            g8T, a4, b4, mybir.AluOpType.add
                                        )
                                        pend_groups.append(g8T)
                        else:
                            emit_psd(pT[:, lo:], lo)
                        # lag group ones-matmuls behind the DVE adds so
                        # the PE never waits; start/stop flags are
                        # counter-driven so emission order is free
                        while len(pend_groups) > 1:
                            emit_psd(pend_groups.pop(0), 0)
                        blk_ctr[0] += 1
                        if (
                            weave_q
                            and blk_ctr[0] % 2 == 0
                            and not (g == n_quads - 1 and h == hl - 1)
                        ):
                            weave_q.pop(0)()

                    LA = 3
                    stage = {}
                    for sk in range(nsk):
                        stage[sk] = emit_score(sk)
                        if sk - LA >= 0:
                            emit_pv(sk - LA, *stage.pop(sk - LA))
                    for sk in range(max(0, nsk - LA), nsk):
                        emit_pv(sk, *stage.pop(sk))
                    while pend_g4:
                        emit_psd(pend_g4.pop(0), 0)
                    while pend_groups:
                        emit_psd(pend_groups.pop(0), 0)

                    # evacuate unnormalized oT (DVE; ACT does exp only in
                    # this phase) right away so the pso psum bank frees a
                    # full sequence early
                    oraw = ph2.tile([P, qw], bf16, tag=f"oraw{h}")
                    nc.vector.tensor_copy(oraw, pso)

                    # denominator chain on DVE (off the PE critical path):
                    # den' = den + (1-m_q); inv = 1/den'; inv *= m_q
                    gsl = slice(g * qw, (g + 1) * qw)
                    dsafe = ph2.tile([1, qw], mybir.dt.float32, tag="dsafe")
                    nc.vector.tensor_tensor(
                        dsafe, psd, dvalrow_sb[:, gsl], mybir.AluOpType.add
                    )
                    rinv = ph2.tile([1, qw], mybir.dt.float32, tag="rinv")
                    nc.vector.reciprocal(rinv, dsafe)
                    inv = ph2.tile([1, qw], bf16, tag=f"inv{h}")
                    nc.vector.tensor_tensor(
                        inv, rinv, mrow_sb[:, gsl], mybir.AluOpType.mult
                    )
                    return oraw, inv

                def emit_flush(g, h, oraw, inv):
                    """Broadcast inv to 128 partitions (PE), evacuate
                    normalized oT, stage the A2A slab; on h==1 trigger the
                    quad's collective + result load."""
                    gsl = slice(g * qw, (g + 1) * qw)
                    bc_ps = ps_s.tile([P, qw], mybir.dt.float32, tag="scT")
                    nc.tensor.matmul(
                        bc_ps, lhsT=ones_row, rhs=inv, start=True, stop=True
                    )
                    bc_sb = ph2.tile([P, qw], bf16, tag="bc_sb")
                    nc.vector.tensor_copy(bc_sb, bc_ps)
                    oTq = ph2.tile([P, qw], bf16, tag=f"oTq{h}")
                    nc.vector.tensor_tensor(
                        oTq, oraw, bc_sb, mybir.AluOpType.mult
                    )
                    nc.vector.tensor_tensor(
                        oTq, oTq, vT[h][:, gsl], mybir.AluOpType.add
                    )
                    p, half = g // 2, g % 2
                    if g >= n_quads - 2:
                        a2a_in_v = a2a_in_q[g].rearrange(
                            "(j e pp) c -> e pp j c", j=n_cores, e=hl
                        )
                        nc.sync.dma_start(
                            a2a_in_v[h],
                            oTq.rearrange("p (j c) -> p j c", j=n_cores),
                        )
                        if h == hl - 1:
                            nc.gpsimd.collective_compute(
                                "AllToAll",
                                mybir.AluOpType.bypass,
                                replica_groups=[list(range(n_cores))],
                                ins=[a2a_in_q[g].opt()],
                                outs=[a2a_out_q[g].opt()],
                            )
                            if half == 0:
                                lhs_tiles[p] = lhsppool.tile(
                                    [P, kd, P], bf16, tag="lhsP", name="lhsP"
                                )
                            a2a_outq_v = a2a_out_q[g].rearrange(
                                "(j e pp) c -> pp (j e) c", j=n_cores, e=hl
                            )
                            csl = slice(
                                half * rows_per_rank, (half + 1) * rows_per_rank
                            )
                            for j4 in range(kd // 4):
                                jsl = slice(j4 * 4, (j4 + 1) * 4)
                                nc.gpsimd.dma_start(
                                    lhs_tiles[p][:, jsl, csl],
                                    a2a_outq_v[:, jsl],
                                )
                        return
                    a2a_in_v = a2a_in[p].rearrange(
                        "(j e pp) (q c) -> e pp j q c", j=n_cores, e=hl, q=2
                    )
                    nc.sync.dma_start(
                        a2a_in_v[h, :, :, half],
                        oTq.rearrange("p (j c) -> p j c", j=n_cores),
                    )
                    if h == hl - 1 and half == 1:
                        nc.gpsimd.collective_compute(
                            "AllToAll",
                            mybir.AluOpType.bypass,
                            replica_groups=[list(range(n_cores))],
                            ins=[a2a_in[p].opt()],
                            outs=[a2a_out[p].opt()],
                        )
                        lhs_tiles[p] = lhsppool.tile(
                            [P, kd, P], bf16, tag="lhsP", name="lhsP"
                        )
                        a2a_out_v = a2a_out[p].rearrange(
                            "(j e pp) c -> pp (j e) c", j=n_cores, e=hl
                        )
                        for j4 in range(kd // 4):
                            jsl = slice(j4 * 4, (j4 + 1) * 4)
                            nc.gpsimd.dma_start(
                                lhs_tiles[p][:, jsl], a2a_out_v[:, jsl]
                            )

                op_state = {}

                def emit_outproj(p, ntiles=None, final=True):
                    """Out-projection for my 128 rows of quad pair p
                    (optionally a subset of 512-wide output tiles)."""
                    if p in op_state:
                        lhsP, partial = op_state.pop(p)
                    else:
                        lhsP = lhs_tiles.pop(p)
                        partial = ph2.tile([P, d], bf16, tag="partial",
                                           name="partial")
                    for ntile in ntiles or range(d // 512):
                        nsl = slice(ntile * 512, (ntile + 1) * 512)
                        pso2 = ps_out.tile([P, 512], mybir.dt.float32, tag="outps")
                        for j in range(kd):
                            nc.tensor.matmul(
                                pso2,
                                lhsT=lhsP[:, j],
                                rhs=wout_sb[:, j, nsl],
                                start=(j == 0),
                                stop=(j == kd - 1),
                            )
                        nc.vector.tensor_copy(partial[:, nsl], pso2)
                        nc.sync.dma_start(
                            out_d[p * P : (p + 1) * P, nsl], partial[:, nsl]
                        )
                    if not final:
                        op_state[p] = (lhsP, partial)

                # main loop: flush of sequence i lags behind the block
                # stream of sequence i+1 so PE never waits on DVE/A2A;
                # outproj(p) is held for two extra flush slots after its
                # second A2A triggers so the collective can complete
                pending = None
                outproj_queue = []  # entries (pair, flush_count_at_append)
                n_flushed = 0
                for g in range(n_quads):
                    for h in range(hl):
                        handles = emit_sdpa_blocks(g, h)
                        if pending is not None:
                            emit_flush(*pending)
                            n_flushed += 1
                            _ages = {0: 7, 1: 4, 2: 99}
                            if outproj_queue and n_flushed >= outproj_queue[0][1] + _ages.get(
                                outproj_queue[0][0], 2
                            ):
                                pq = outproj_queue.pop(0)[0]
                                if pq == 0:
                                    push_outproj_weave(pq)
                                else:
                                    emit_outproj(pq)
                            pg, phh = pending[0], pending[1]
                            if phh == hl - 1 and pg % 2 == 1:
                                outproj_queue.append((pg // 2, n_flushed))
                        pending = (g, h, *handles)
                emit_flush(*pending)
                while weave_q:
                    weave_q.pop(0)()
                while outproj_queue:
                    pq = outproj_queue.pop(0)[0]
                    if pq != n_pairs - 1:
                        emit_outproj(pq)
                emit_outproj(n_pairs - 1)

    nc.compile()
    return nc


def prepare_in_maps(x, W_qkv, W_out, cos, sin, mask, n_cores=N_CORES, hl=H // N_CORES):
    """Host-side sharding. Returns list of per-core input dicts."""
    t, d = x.shape
    x = np.asarray(x, dtype=BF16)
    W_qkv = np.asarray(W_qkv, dtype=BF16)
    W_out = np.asarray(W_out, dtype=BF16)
    cos = np.asarray(cos, dtype=np.float32)
    sin = np.asarray(sin, dtype=np.float32)
    m = np.asarray(mask, dtype=bool)

    xT = np.ascontiguousarray(x.T)
    cosT = np.ascontiguousarray(cos.T)
    sign = np.where(np.arange(DH) < DH // 2, -1.0, 1.0).astype(np.float32)
    ssinT = np.ascontiguousarray(sin.T * sign[:, None])

    mf = m.astype(np.float32)
    rqT = np.ascontiguousarray(
        np.where(mf, np.float32(0.0), np.float32(-1e9)).reshape(-1, DH).T
    )
    dvalB = np.ascontiguousarray(
        np.broadcast_to((1.0 - mf).astype(BF16)[None, :], (DH, t))
    )
    mrow = np.ascontiguousarray(mf.astype(BF16).reshape(1, t))
    dvalrow = np.ascontiguousarray((1.0 - mf).astype(BF16).reshape(1, t))
    cmask128 = (np.arange(DH)[None, :] >= np.arange(DH)[:, None]).astype(BF16)

    n_heads = W_qkv.shape[1] // 3 // DH
    in_maps = []
    for c in range(n_cores):
        hs = [c * hl + i for i in range(hl)]
        cols = [W_qkv[:, (s * n_heads + h) * DH : (s * n_heads + h) * DH + DH]
                for s in range(3) for h in hs]
        wqkv_c = np.concatenate(cols, axis=1)  # [d, 768]
        # -> [p, c, k, j] partition-major for efficient weight DMA
        wqkv_c = np.ascontiguousarray(
            wqkv_c.reshape(16, 128, 6, 128).transpose(1, 2, 0, 3).reshape(128, -1)
        )
        in_maps.append(
            {
                "xT": xT,
                "wqkv": wqkv_c,
                "wout": W_out,
                "cosT": cosT,
                "ssinT": ssinT,
                "rqT": rqT,
                "dvalB": dvalB,
                "mrow": mrow,
                "dvalrow": dvalrow,
                "cmask128": cmask128,
            }
        )
    return in_maps


_CACHED_NC = None


def assemble(results, t=T, d=D, n_cores=N_CORES):
    """Reassemble per-core A2A row shards into the full output.
    Core r's out row g*64 + i is global row 512*g + 64*r + i."""
    qw = 512
    rows = qw // n_cores  # 64
    out = np.empty((t, d), dtype=BF16)
    for r in range(n_cores):
        oc = np.asarray(results[r]["out"])
        if oc.dtype != BF16:
            oc = oc.view(BF16)
        for g in range(t // qw):
            out[qw * g + rows * r : qw * g + rows * (r + 1)] = oc[
                rows * g : rows * (g + 1)
            ]
    return out


def kernel(x, W_qkv, W_out, cos, sin, mask):
    """Full inputs in, full output out. Shards across 8 NeuronCores."""
    global _CACHED_NC
    from concourse import bass_utils

    if _CACHED_NC is None:
        _CACHED_NC = build_nc()
    nc = _CACHED_NC

    in_maps = prepare_in_maps(x, W_qkv, W_out, cos, sin, mask)
    res = bass_utils.run_bass_kernel_spmd(
        nc, in_maps, core_ids=list(range(N_CORES))
    )
    return assemble(res.results)


# revision 54
# speedup vs baseline: 1.2769x; 1.2769x over previous
"""Distributed Trainium2 attention kernel (8 NeuronCores, head tensor-parallel).

Reference semantics (T=4096, D=2048, H=16, DH=128):
  qkv = bf16(x @ W_qkv); q,k,v per head; RoPE(split-half) on q,k;
  mask = ((m_q & m_k) | eye) & causal; softmax(q k^T / sqrt(DH) masked);
  out = bf16((probs @ v) @ W_out)

Sharding: head tensor-parallel for qkv+SDPA (core c owns heads 2c, 2c+1),
then an AllToAll redistributes the small per-head attention outputs o so
that each core owns 64 output ROWS per 512-query quad (512 rows total)
and computes the full out-projection locally against a replicated W_out.
This moves 8x fewer bytes than reduce-scattering output partials.

Device-side layout choices:
  - x passed as xT [D, T] so the D contraction dim is the partition dim.
  - q,k computed weight-stationary -> born transposed [DH, T]; v
    transposed back to natural [T, DH] via PE (PV lhsT layout).
  - RoPE: partition-rotate by 64 via two SBUF->SBUF DMAs, sign folded
    into a host-precomputed ssinT table; combine on DVE.
  - SDPA in transposed-scores form: scoresT[k, q] tiles over 512-query
    quads; exp (no max-subtraction; scores are O(5)) evacuates the
    scores psum straight into the PV rhs; key padding mask folded into
    the exp bias (per-k = per-partition); within-block causal via one
    precomputed 0/1 [128,128] multiply; softmax denominators via a
    ones-column matmul.
  - Denominators: non-diagonal exp blocks are tree-summed on DVE in
    bf16 (pairs -> 4-groups -> 8-groups, ~0.3% on the sum) so the PE
    ones-matmul runs once per up-to-8 blocks; diagonal blocks keep
    per-block ones-matmuls (they carry the partial-column offsets).
  - Normalization fused into the oT evacuation: inv = m_q/(den+(1-m_q))
    computed on one partition, broadcast to all 128 partitions with a
    K=1 ones-outer-product matmul on PE, then oT = oraw*bc + vT*(1-m_q)
    (vT premultiplied by (1-m_q) once). Masked queries thereby attend
    only to themselves; out-projection needs no per-tile scaling.
  - Engine split in phase 2: ACT does exp only; DVE does all psum
    evacuations, the denominator tree and the inv chain; bc matmuls and
    evacuations lag one head-sequence behind the PE block stream so the
    PE never waits on the DVE chain.
  - A2A per quad PAIR (512KB) for pairs 0-2; the final pair is split
    into per-quad A2As so quad 6's half flies during quad 7's SDPA and
    the exposed tail collective carries only 256KB.  Two tiny warm-up
    A2As (kernel start + phase boundary) absorb CC-path init and rank
    skew.  Collective-dependent loads ride the GpSimd DMA queue so they
    cannot head-of-line-block the Sync queue; bulk input loads also use
    the GpSimd SWDGE ring, which sustains far higher bandwidth than the
    HWDGE rings.
  - Out-projections are lag-scheduled behind their A2As.  The exp on
    ACT (686ns/block) slightly outpaces the PE block stream (~620ns),
    so pair 0's projection is WOVEN into the SDPA stream one matmul per
    two blocks, executing inside that per-block PE slack; pair 2's
    projection is deferred entirely into the final A2A's rendezvous
    window; pair 3 is the unavoidable post-collective tail.
"""

import os
import sys

import numpy as np

sys.path.insert(0, "/opt/trn_rl_repo")

import ml_dtypes

BF16 = ml_dtypes.bfloat16

# problem constants (hardcoded per harness contract)
T, D, H, DH = 4096, 2048, 16, 128
N_CORES = 8
ROPE_BASE = 10000.0


def build_nc(
    t=T,
    d=D,
    n_cores=N_CORES,
    hl=H // N_CORES,  # heads per core
    tch=512,  # qkv t-chunk
):
    import concourse.bass as bass
    import concourse.mybir as mybir
    import concourse.tile as tile
    from concourse import bacc
    from concourse.masks import make_identity

    f32 = mybir.dt.float32
    bf16 = mybir.dt.bfloat16

    P = 128
    kd = d // P  # contraction chunks for qkv
    qb_n = t // P  # q-blocks of 128 rows
    nt = t // tch  # t-chunks in qkv phase
    qw = 512  # queries per quad
    n_quads = t // qw
    qb_per_quad = qw // P  # 4
    rows_per_rank = qw // n_cores  # 64 rows each rank owns per quad
    n_pairs = n_quads // 2
    t_out = t // n_cores  # output rows per core
    scale = 1.0 / np.sqrt(DH)

    nc = bacc.Bacc(
        "TRN2", target_bir_lowering=False, debug=False, num_devices=n_cores
    )

    xT = nc.dram_tensor("xT", [d, t], bf16, kind="ExternalInput").ap()
    # wqkv pre-transposed on host to partition-major [P, c, k, j] so
    # per-channel weight DMAs have 4KB-contiguous lines per partition
    wqkv = nc.dram_tensor("wqkv", [P, 3 * hl * kd * P], bf16, kind="ExternalInput").ap()
    wout_d = nc.dram_tensor("wout", [d, d], bf16, kind="ExternalInput").ap()
    cosT_d = nc.dram_tensor("cosT", [P, t], f32, kind="ExternalInput").ap()
    ssinT_d = nc.dram_tensor("ssinT", [P, t], f32, kind="ExternalInput").ap()
    # rqT[p, qb] = 0 if mask[qb*128+p] else -1e9 (folded into exp bias)
    rqT_d = nc.dram_tensor("rqT", [P, qb_n], f32, kind="ExternalInput").ap()
    # dvalB[p, q] = 1 - mask[q], broadcast to all partitions
    dvalB_d = nc.dram_tensor("dvalB", [P, t], bf16, kind="ExternalInput").ap()
    # mrow[0, q] = mask[q] ; dvalrow[0, q] = 1 - mask[q]
    mrow_d = nc.dram_tensor("mrow", [1, t], bf16, kind="ExternalInput").ap()
    dvalrow_d = nc.dram_tensor("dvalrow", [1, t], bf16, kind="ExternalInput").ap()
    # cmask128[p, j] = 1 if j >= p else 0 (within-block causal, T-orientation)
    cmask128_d = nc.dram_tensor("cmask128", [P, P], bf16, kind="ExternalInput").ap()
    out_d = nc.dram_tensor("out", [t_out, d], bf16, kind="ExternalOutput").ap()

    with tile.TileContext(nc) as tc:
        with tc.tile_pool(name="persist", bufs=1) as persist:
            ident = persist.tile([P, P], bf16, name="ident")
            ones_col = persist.tile([P, 1], bf16, name="ones_col")
            nc.vector.memset(ones_col, 1.0)
            ones_row = persist.tile([1, P], bf16, name="ones_row")
            nc.vector.memset(ones_row, 1.0)
            rqT_sb = persist.tile([P, qb_n], f32, name="rqT_sb")
            cm128_sb = persist.tile([P, P], bf16, name="cm128_sb")
            mrow_sb = persist.tile([1, t], bf16, name="mrow_sb")
            dvalrow_sb = persist.tile([1, t], bf16, name="dvalrow_sb")

            def load_masks():
                # deferred: these tiny loads would otherwise delay the x
                # stream at the head of the gpsimd DMA ring
                nc.gpsimd.dma_start(rqT_sb, rqT_d)
                nc.gpsimd.dma_start(cm128_sb, cmask128_d)
                nc.gpsimd.dma_start(mrow_sb, mrow_d)
                nc.gpsimd.dma_start(dvalrow_sb, dvalrow_d)

            with tc.tile_pool(name="dram_warm", bufs=1, space="DRAM") as dwarm:
                warm_in = dwarm.tile([n_cores * 16, 16], bf16, name="cc_warm_in")
                warm_out = dwarm.tile([n_cores * 16, 16], bf16, name="cc_warm_out")

            # per-head persistent activations
            qT = [persist.tile([P, t], bf16, name=f"qT{h}") for h in range(hl)]
            kT = [persist.tile([P, t], bf16, name=f"kT{h}") for h in range(hl)]
            vT = [persist.tile([P, t], bf16, name=f"vT{h}") for h in range(hl)]
            v_nat = [
                persist.tile([P, qb_n, P], bf16, name=f"vnat{h}") for h in range(hl)
            ]

            # ---------------- phase 1: qkv + rope + v transpose ----------
            with (
                tc.tile_pool(name="wq", bufs=1) as wqpool,
                tc.tile_pool(name="cs", bufs=1) as cspool,
                tc.tile_pool(name="ph1", bufs=2) as ph1,
                tc.tile_pool(name="ps_qkv", bufs=1, space="PSUM") as ps_qkv,
                tc.tile_pool(name="ps_aux", bufs=2, space="PSUM") as ps_aux,
            ):
                cosT_sb = cspool.tile([P, t], f32, name="cosT_sb")
                ssinT_sb = cspool.tile([P, t], f32, name="ssinT_sb")
                dvalB_sb = cspool.tile([P, t], bf16, name="dvalB_sb")
                # rope tables on the gpsimd (SWDGE) ring, first two
                # t-chunks upfront, the rest paced inside the loop so they
                # don't steal DMA bandwidth from the x stream at startup
                def load_tables(tc_i):
                    tsl = slice(tc_i * tch, (tc_i + 1) * tch)
                    nc.gpsimd.dma_start(cosT_sb[:, tsl], cosT_d[:, tsl])
                    nc.gpsimd.dma_start(ssinT_sb[:, tsl], ssinT_d[:, tsl])



                # fire a tiny throwaway AllToAll after the bulk loads: the
                # first collective pays the CC-path init + rank-sync cost
                # (~100us on the baseline's first ReduceScatter), which
                # this hides under the qkv phase without delaying the
                # gpsimd DMA ring; a second one at the end of phase 1
                # re-syncs rank skew just before the real A2As
                nc.gpsimd.collective_compute(
                    "AllToAll",
                    mybir.AluOpType.bypass,
                    replica_groups=[list(range(n_cores))],
                    ins=[warm_in.opt()],
                    outs=[warm_out.opt()],
                )

                # weights channel-major so channel 0's 16 k-chunks land
                # first and the very first matmul group isn't DMA-starved
                wq_sb = wqpool.tile([P, 3 * hl, kd, P], bf16, name="wq_sb")
                wqkv_r = wqkv.rearrange("p (c kd j) -> p c kd j", kd=kd, j=P)
                xT_r = xT.rearrange("(kd p) x -> p kd x", p=P)

                xts = {}

                def load_xt(tc_i):
                    # x rides the gpsimd (SWDGE) ring, which empirically
                    # sustains ~250GB/s vs ~50GB/s on the HWDGE rings
                    xt = ph1.tile([P, kd, tch], bf16, tag="xt")
                    tsl = slice(tc_i * tch, (tc_i + 1) * tch)
                    gw = 2 if tc_i == 0 else 4
                    for k4 in range(kd // gw):
                        ksl = slice(k4 * gw, (k4 + 1) * gw)
                        nc.gpsimd.dma_start(xt[:, ksl], xT_r[:, ksl, tsl])
                    nc.gpsimd.dma_start(dvalB_sb[:, tsl], dvalB_d[:, tsl])
                    if tc_i == 0:
                        # weights ride the sync ring, streaming in
                        # parallel with the x chunks on the gpsimd ring
                        for c in range(3 * hl):
                            nc.sync.dma_start(wq_sb[:, c], wqkv_r[:, c])
                    xts[tc_i] = xt

                pending_vt = []

                def flush_vt():
                    h, b0, b1 = pending_vt.pop(0)
                    for b in range(b0, b1):
                        pst = ps_aux.tile([P, P], bf16, tag="aux")
                        nc.tensor.transpose(
                            pst, vT[h][:, b * P : (b + 1) * P], ident
                        )
                        nc.scalar.copy(v_nat[h][:, b], pst)
                    # vT only feeds the masked-query blend once transposed:
                    # premultiply this chunk by (1 - m_q) in place
                    csl = slice(b0 * P, b1 * P)
                    nc.vector.tensor_tensor(
                        vT[h][:, csl], vT[h][:, csl], dvalB_sb[:, csl],
                        mybir.AluOpType.mult,
                    )

                load_xt(0)
                load_tables(0)
                load_tables(1)
                # identity for the PE transposes (gpsimd memset+select is
                # slow on Q7 -- keep it behind the startup x/weight DMAs)
                make_identity(nc, ident)
                for tc_i in range(nt):
                    tsl = slice(tc_i * tch, (tc_i + 1) * tch)
                    if tc_i + 1 < nt:
                        load_xt(tc_i + 1)  # prefetch one chunk ahead
                    if tc_i + 2 < nt:
                        load_tables(tc_i + 2)
                    if tc_i == 2:
                        load_masks()
                    xt = xts.pop(tc_i)
                    for c in range(3 * hl):  # q0,q1,k0,k1,v0,v1
                        ps = ps_qkv.tile([P, tch], mybir.dt.float32, tag=f"ps{c}")
                        for k in range(kd):
                            nc.tensor.matmul(
                                ps,
                                lhsT=wq_sb[:, c, k],
                                rhs=xt[:, k],
                                start=(k == 0),
                                stop=(k == kd - 1),
                            )
                        if pending_vt:
                            # v transposes lag one channel behind so the PE
                            # never waits on the vT psum->sbuf cast
                            flush_vt()
                        if c < 2 * hl:  # q or k: cast, rotate, rope-combine
                            dst = qT[c] if c < hl else kT[c - hl]
                            qbf = ph1.tile([P, tch], bf16, tag="qbf")
                            nc.scalar.copy(qbf, ps)
                            # rotate-half: partition shift by 64 via two
                            # SBUF->SBUF DMAs (keeps PE free; scalar queue
                            # so the sync queue keeps streaming x chunks)
                            shift = ph1.tile([P, tch], bf16, tag="shift")
                            nc.scalar.dma_start(shift[0:64], qbf[64:128])
                            nc.scalar.dma_start(shift[64:128], qbf[0:64])
                            t1 = ph1.tile([P, tch], f32, tag="t1")
                            nc.vector.tensor_tensor(
                                t1, qbf, cosT_sb[:, tsl], mybir.AluOpType.mult
                            )
                            t2 = ph1.tile([P, tch], f32, tag="t2")
                            nc.vector.tensor_tensor(
                                t2, shift, ssinT_sb[:, tsl], mybir.AluOpType.mult
                            )
                            nc.vector.tensor_tensor(
                                dst[:, tsl], t1, t2, mybir.AluOpType.add
                            )
                        else:  # v: cast; queue this chunk's transposes
                            h = c - 2 * hl
                            nc.scalar.copy(vT[h][:, tsl], ps)
                            pending_vt.append(
                                (h, tc_i * tch // P, (tc_i + 1) * tch // P)
                            )
                while pending_vt:
                    flush_vt()
                nc.gpsimd.collective_compute(
                    "AllToAll",
                    mybir.AluOpType.bypass,
                    replica_groups=[list(range(n_cores))],
                    ins=[warm_in.opt()],
                    outs=[warm_out.opt()],
                )

            # ---------------- phase 2: SDPA + A2A + out-proj -------------
            with (
                tc.tile_pool(name="w2", bufs=1) as w2pool,
                tc.tile_pool(name="ph2", bufs=2) as ph2,
                tc.tile_pool(name="pt", bufs=5) as ptpool,
                tc.tile_pool(name="pair", bufs=4) as pairpool,
                tc.tile_pool(name="lhsp", bufs=3) as lhsppool,
                tc.tile_pool(name="dram", bufs=1, space="DRAM") as dram,
                tc.tile_pool(name="ps_s", bufs=3, space="PSUM") as ps_s,
                tc.tile_pool(name="ps_o", bufs=2, space="PSUM") as ps_o,
                tc.tile_pool(name="ps_d", bufs=1, space="PSUM") as ps_d,
                tc.tile_pool(name="ps_out", bufs=2, space="PSUM") as ps_out,
            ):
                wout_sb = w2pool.tile([P, kd, d], bf16, name="wout_sb")
                nc.gpsimd.dma_start(
                    wout_sb, wout_d.rearrange("(h p) x -> p h x", p=P)
                )

                # one A2A per quad PAIR (512KB) instead of per quad:
                # halves the latency-bound collective-pipe occupancy.
                # The final pair is split back into per-quad A2As so quad
                # 6's half flies during quad 7's SDPA and the exposed
                # tail collective carries only 256KB.
                a2a_in = [
                    dram.tile([hl * n_cores * P, 2 * rows_per_rank], bf16,
                              name=f"a2a_in{p}")
                    for p in range(n_pairs - 1)
                ]
                a2a_out = [
                    dram.tile([hl * n_cores * P, 2 * rows_per_rank], bf16,
                              name=f"a2a_out{p}")
                    for p in range(n_pairs - 1)
                ]
                a2a_in_q = {
                    g: dram.tile([hl * n_cores * P, rows_per_rank], bf16,
                                 name=f"a2a_inq{g}")
                    for g in (n_quads - 2, n_quads - 1)
                }
                a2a_out_q = {
                    g: dram.tile([hl * n_cores * P, rows_per_rank], bf16,
                                 name=f"a2a_outq{g}")
                    for g in (n_quads - 2, n_quads - 1)
                }

                lhs_tiles = {}
                weave_q = []
                blk_ctr = [0]

                def push_outproj_weave(p):
                    """Queue pair p's out-projection as fine-grained items
                    woven one-per-two SDPA blocks, consuming the PE slack
                    left by the exp-bound block stream."""
                    lhsP = lhs_tiles.pop(p)
                    partial = ph2.tile([P, d], bf16, tag="partial",
                                       name="partial")
                    open_ps = {}
                    for ntile in range(d // 512):
                        nsl = slice(ntile * 512, (ntile + 1) * 512)
                        for j in range(kd):
                            def mm(j=j, ntile=ntile, nsl=nsl):
                                if j == 0:
                                    open_ps[ntile] = ps_out.tile(
                                        [P, 512], mybir.dt.float32,
                                        tag="outps", name="outps_w"
                                    )
                                nc.tensor.matmul(
                                    open_ps[ntile],
                                    lhsT=lhsP[:, j],
                                    rhs=wout_sb[:, j, nsl],
                                    start=(j == 0),
                                    stop=(j == kd - 1),
                                )
                            weave_q.append(mm)
                        def evac(ntile=ntile, nsl=nsl):
                            nc.vector.tensor_copy(
                                partial[:, nsl], open_ps.pop(ntile)
                            )
                            nc.sync.dma_start(
                                out_d[p * P : (p + 1) * P, nsl], partial[:, nsl]
                            )
                        weave_q.append(evac)

                def emit_sdpa_blocks(g, h):
                    """PE block stream for one (quad, head): scores, exp,
                    PV + denominator accumulate.  Returns psum handles.

                    Denominator: non-diagonal blocks are pair-summed on DVE
                    (bf16, ~0.1% on the sum) so the PE ones-matmul runs once
                    per pair; diagonal blocks keep per-block ones-matmuls."""
                    nsk = (g + 1) * qb_per_quad
                    diag_start = g * qb_per_quad
                    pso = ps_o.tile([P, qw], mybir.dt.float32, tag="pso")
                    psd = ps_d.tile([1, qw], mybir.dt.float32, tag="psd")
                    # non-diag blocks are tree-summed on DVE (pairs ->
                    # 4-groups -> 8-groups) so the PE ones-matmul runs once
                    # per up-to-8 blocks; ~0.3% on the denominator, far
                    # inside the error budget
                    ndiag = diag_start
                    n_psd = ndiag // 8 + (1 if ndiag % 8 else 0) + qb_per_quad
                    psd_state = [0]
                    pend_pairs = []
                    pend_g4 = []
                    pend_groups = []

                    def emit_psd(rhs_ap, lo):
                        nc.tensor.matmul(
                            psd[:, lo:],
                            lhsT=ones_col,
                            rhs=rhs_ap,
                            start=(psd_state[0] == 0),
                            stop=(psd_state[0] == n_psd - 1),
                        )
                        psd_state[0] += 1

                    def emit_score(sk):
                        br = sk - diag_start  # >=0 in diag region
                        lo = br * P if br >= 0 else 0
                        psT = ps_s.tile([P, qw], mybir.dt.float32, tag="scT")
                        nc.tensor.matmul(
                            psT[:, lo:],
                            lhsT=kT[h][:, sk * P : (sk + 1) * P],
                            rhs=qT[h][:, g * qw + lo : (g + 1) * qw],
                            start=True,
                            stop=True,
                        )
                        pT = ptpool.tile([P, qw], bf16, tag="pT")
                        nc.scalar.activation(
                            pT[:, lo:],
                            psT[:, lo:],
                            mybir.ActivationFunctionType.Exp,
                            scale=float(scale),
                            bias=rqT_sb[:, sk : sk + 1],
                        )
                        if br >= 0:
                            nc.vector.tensor_tensor(
                                pT[:, lo : lo + P],
                                pT[:, lo : lo + P],
                                cm128_sb,
                                mybir.AluOpType.mult,
                            )
                        return pT, lo

                    prev_pT = [None]

                    def emit_pv(sk, pT, lo):
                        nc.tensor.matmul(
                            pso[:, lo:],
                            lhsT=v_nat[h][:, sk],
                            rhs=pT[:, lo:],
                            start=(sk == 0),
                            stop=(sk == nsk - 1),
                        )
                        if sk < diag_start:
                            if sk % 2 == 0:
                                prev_pT[0] = pT
                            else:
                                pairT = pairpool.tile([P, qw], bf16, tag="pairT")
                                nc.vector.tensor_tensor(
                                    pairT, prev_pT[0], pT, mybir.AluOpType.add
                                )
                                pend_pairs.append(pairT)
                                if len(pend_pairs) == 2:
                                    b = pend_pairs.pop()
                                    a = pend_pairs.pop()
                                    grpT = pairpool.tile(
                                        [P, qw], bf16, tag="grpT"
                                    )
                                    nc.vector.tensor_tensor(
                                        grpT, a, b, mybir.AluOpType.add
                                    )
                                    pend_g4.append(grpT)
                                    if len(pend_g4) == 2:
                                        b4 = pend_g4.pop()
                                        a4 = pend_g4.pop()
                                        g8T = pairpool.tile(
                                            [P, qw], bf16, tag="g8T"
                                        )
                                        nc.vector.tensor_tensor(
                                            g8T, a4, b4, mybir.AluOpType.add
                                        )
                                        pend_groups.append(g8T)
                        else:
                            emit_psd(pT[:, lo:], lo)
                        # lag group ones-matmuls behind the DVE adds so
                        # the PE never waits; start/stop flags are
                        # counter-driven so emission order is free
                        while len(pend_groups) > 1:
                            emit_psd(pend_groups.pop(0), 0)
                        blk_ctr[0] += 1
                        if (
                            weave_q
                            and blk_ctr[0] % 2 == 0
                            and not (g == n_quads - 1 and h == hl - 1)
                        ):
                            weave_q.pop(0)()

                    LA = 3
                    stage = {}
                    for sk in range(nsk):
                        stage[sk] = emit_score(sk)
                        if sk - LA >= 0:
                            emit_pv(sk - LA, *stage.pop(sk - LA))
                    for sk in range(max(0, nsk - LA), nsk):
                        emit_pv(sk, *stage.pop(sk))
                    while pend_g4:
                        emit_psd(pend_g4.pop(0), 0)
                    while pend_groups:
                        emit_psd(pend_groups.pop(0), 0)

                    # evacuate unnormalized oT (DVE; ACT does exp only in
                    # this phase) right away so the pso psum bank frees a
                    # full sequence early
                    oraw = ph2.tile([P, qw], bf16, tag=f"oraw{h}")
                    nc.vector.tensor_copy(oraw, pso)

                    # denominator chain on DVE (off the PE critical path):
                    # den' = den + (1-m_q); inv = 1/den'; inv *= m_q
                    gsl = slice(g * qw, (g + 1) * qw)
                    dsafe = ph2.tile([1, qw], mybir.dt.float32, tag="dsafe")
                    nc.vector.tensor_tensor(
                        dsafe, psd, dvalrow_sb[:, gsl], mybir.AluOpType.add
                    )
                    rinv = ph2.tile([1, qw], mybir.dt.float32, tag="rinv")
                    nc.vector.reciprocal(rinv, dsafe)
                    inv = ph2.tile([1, qw], bf16, tag=f"inv{h}")
                    nc.vector.tensor_tensor(
                        inv, rinv, mrow_sb[:, gsl], mybir.AluOpType.mult
                    )
                    return oraw, inv

                def emit_flush(g, h, oraw, inv):
                    """Broadcast inv to 128 partitions (PE), evacuate
                    normalized oT, stage the A2A slab; on h==1 trigger the
                    quad's collective + result load."""
                    gsl = slice(g * qw, (g + 1) * qw)
                    bc_ps = ps_s.tile([P, qw], mybir.dt.float32, tag="scT")
                    nc.tensor.matmul(
                        bc_ps, lhsT=ones_row, rhs=inv, start=True, stop=True
                    )
                    bc_sb = ph2.tile([P, qw], bf16, tag="bc_sb")
                    nc.vector.tensor_copy(bc_sb, bc_ps)
                    oTq = ph2.tile([P, qw], bf16, tag=f"oTq{h}")
                    nc.vector.tensor_tensor(
                        oTq, oraw, bc_sb, mybir.AluOpType.mult
                    )
                    nc.vector.tensor_tensor(
                        oTq, oTq, vT[h][:, gsl], mybir.AluOpType.add
                    )
                    p, half = g // 2, g % 2
                    if g >= n_quads - 2:
                        a2a_in_v = a2a_in_q[g].rearrange(
                            "(j e pp) c -> e pp j c", j=n_cores, e=hl
                        )
                        nc.sync.dma_start(
                            a2a_in_v[h],
                            oTq.rearrange("p (j c) -> p j c", j=n_cores),
                        )
                        if h == hl - 1:
                            nc.gpsimd.collective_compute(
                                "AllToAll",
                                mybir.AluOpType.bypass,
                                replica_groups=[list(range(n_cores))],
                                ins=[a2a_in_q[g].opt()],
                                outs=[a2a_out_q[g].opt()],
                            )
                            if half == 0:
                                lhs_tiles[p] = lhsppool.tile(
                                    [P, kd, P], bf16, tag="lhsP", name="lhsP"
                                )
                            a2a_outq_v = a2a_out_q[g].rearrange(
                                "(j e pp) c -> pp (j e) c", j=n_cores, e=hl
                            )
                            csl = slice(
                                half * rows_per_rank, (half + 1) * rows_per_rank
                            )
                            for j4 in range(kd // 4):
                                jsl = slice(j4 * 4, (j4 + 1) * 4)
                                nc.gpsimd.dma_start(
                                    lhs_tiles[p][:, jsl, csl],
                                    a2a_outq_v[:, jsl],
                                )
                        return
                    a2a_in_v = a2a_in[p].rearrange(
                        "(j e pp) (q c) -> e pp j q c", j=n_cores, e=hl, q=2
                    )
                    nc.sync.dma_start(
                        a2a_in_v[h, :, :, half],
                        oTq.rearrange("p (j c) -> p j c", j=n_cores),
                    )
                    if h == hl - 1 and half == 1:
                        nc.gpsimd.collective_compute(
                            "AllToAll",
                            mybir.AluOpType.bypass,
                            replica_groups=[list(range(n_cores))],
                            ins=[a2a_in[p].opt()],
                            outs=[a2a_out[p].opt()],
                        )
                        lhs_tiles[p] = lhsppool.tile(
                            [P, kd, P], bf16, tag="lhsP", name="lhsP"
                        )
                        a2a_out_v = a2a_out[p].rearrange(
                            "(j e pp) c -> pp (j e) c", j=n_cores, e=hl
                        )
                        for j4 in range(kd // 4):
                            jsl = slice(j4 * 4, (j4 + 1) * 4)
                            nc.gpsimd.dma_start(
                                lhs_tiles[p][:, jsl], a2a_out_v[:, jsl]
                            )

                op_state = {}

                def emit_outproj(p, ntiles=None, final=True):
                    """Out-projection for my 128 rows of quad pair p
                    (optionally a subset of 512-wide output tiles)."""
                    if p in op_state:
                        lhsP, partial = op_state.pop(p)
                    else:
                        lhsP = lhs_tiles.pop(p)
                        partial = ph2.tile([P, d], bf16, tag="partial",
                                           name="partial")
                    for ntile in ntiles or range(d // 512):
                        nsl = slice(ntile * 512, (ntile + 1) * 512)
                        pso2 = ps_out.tile([P, 512], mybir.dt.float32, tag="outps")
                        for j in range(kd):
                            nc.tensor.matmul(
                                pso2,
                                lhsT=lhsP[:, j],
                                rhs=wout_sb[:, j, nsl],
                                start=(j == 0),
                                stop=(j == kd - 1),
                            )
                        nc.vector.tensor_copy(partial[:, nsl], pso2)
                        nc.sync.dma_start(
                            out_d[p * P : (p + 1) * P, nsl], partial[:, nsl]
                        )
                    if not final:
                        op_state[p] = (lhsP, partial)

                # main loop: flush of sequence i lags behind the block
                # stream of sequence i+1 so PE never waits on DVE/A2A;
                # outproj(p) is held for two extra flush slots after its
                # second A2A triggers so the collective can complete
                pending = None
                outproj_queue = []  # entries (pair, flush_count_at_append)
                n_flushed = 0
                for g in range(n_quads):
                    for h in range(hl):
                        handles = emit_sdpa_blocks(g, h)
                        if pending is not None:
                            emit_flush(*pending)
                            n_flushed += 1
                            _ages = {0: 7, 1: 4, 2: 99}
                            if outproj_queue and n_flushed >= outproj_queue[0][1] + _ages.get(
                                outproj_queue[0][0], 2
                            ):
                                pq = outproj_queue.pop(0)[0]
                                if pq == 0:
                                    push_outproj_weave(pq)
                                else:
                                    emit_outproj(pq)
                            pg, phh = pending[0], pending[1]
                            if phh == hl - 1 and pg % 2 == 1:
                                outproj_queue.append((pg // 2, n_flushed))
                        pending = (g, h, *handles)
                emit_flush(*pending)
                while weave_q:
                    weave_q.pop(0)()
                while outproj_queue:
                    pq = outproj_queue.pop(0)[0]
                    if pq != n_pairs - 1:
                        emit_outproj(pq)
                emit_outproj(n_pairs - 1)

    nc.compile()
    return nc


def prepare_in_maps(x, W_qkv, W_out, cos, sin, mask, n_cores=N_CORES, hl=H // N_CORES):
    """Host-side sharding. Returns list of per-core input dicts."""
    t, d = x.shape
    x = np.asarray(x, dtype=BF16)
    W_qkv = np.asarray(W_qkv, dtype=BF16)
    W_out = np.asarray(W_out, dtype=BF16)
    cos = np.asarray(cos, dtype=np.float32)
    sin = np.asarray(sin, dtype=np.float32)
    m = np.asarray(mask, dtype=bool)

    xT = np.ascontiguousarray(x.T)
    cosT = np.ascontiguousarray(cos.T)
    sign = np.where(np.arange(DH) < DH // 2, -1.0, 1.0).astype(np.float32)
    ssinT = np.ascontiguousarray(sin.T * sign[:, None])

    mf = m.astype(np.float32)
    rqT = np.ascontiguousarray(
        np.where(mf, np.float32(0.0), np.float32(-1e9)).reshape(-1, DH).T
    )
    dvalB = np.ascontiguousarray(
        np.broadcast_to((1.0 - mf).astype(BF16)[None, :], (DH, t))
    )
    mrow = np.ascontiguousarray(mf.astype(BF16).reshape(1, t))
    dvalrow = np.ascontiguousarray((1.0 - mf).astype(BF16).reshape(1, t))
    cmask128 = (np.arange(DH)[None, :] >= np.arange(DH)[:, None]).astype(BF16)

    n_heads = W_qkv.shape[1] // 3 // DH
    in_maps = []
    for c in range(n_cores):
        hs = [c * hl + i for i in range(hl)]
        cols = [W_qkv[:, (s * n_heads + h) * DH : (s * n_heads + h) * DH + DH]
                for s in range(3) for h in hs]
        wqkv_c = np.concatenate(cols, axis=1)  # [d, 768]
        # -> [p, c, k, j] partition-major for efficient weight DMA
        wqkv_c = np.ascontiguousarray(
            wqkv_c.reshape(16, 128, 6, 128).transpose(1, 2, 0, 3).reshape(128, -1)
        )
        in_maps.append(
            {
                "xT": xT,
                "wqkv": wqkv_c,
                "wout": W_out,
                "cosT": cosT,
                "ssinT": ssinT,
                "rqT": rqT,
                "dvalB": dvalB,
                "mrow": mrow,
                "dvalrow": dvalrow,
                "cmask128": cmask128,
            }
        )
    return in_maps


_CACHED_NC = None


def assemble(results, t=T, d=D, n_cores=N_CORES):
    """Reassemble per-core A2A row shards into the full output.
    Core r's out row g*64 + i is global row 512*g + 64*r + i."""
    qw = 512
    rows = qw // n_cores  # 64
    out = np.empty((t, d), dtype=BF16)
    for r in range(n_cores):
        oc = np.asarray(results[r]["out"])
        if oc.dtype != BF16:
            oc = oc.view(BF16)
        for g in range(t // qw):
            out[qw * g + rows * r : qw * g + rows * (r + 1)] = oc[
                rows * g : rows * (g + 1)
            ]
    return out


def kernel(x, W_qkv, W_out, cos, sin, mask):
    """Full inputs in, full output out. Shards across 8 NeuronCores."""
    global _CACHED_NC
    from concourse import bass_utils

    if _CACHED_NC is None:
        _CACHED_NC = build_nc()
    nc = _CACHED_NC

    in_maps = prepare_in_maps(x, W_qkv, W_out, cos, sin, mask)
    res = bass_utils.run_bass_kernel_spmd(
        nc, in_maps, core_ids=list(range(N_CORES))
    )
    return assemble(res.results)
